# revision 24
# baseline (speedup 1.0000x reference)
"""Trainium2 Bass kernel for int8 quantized linear with requantization.

Computes, for x:int8[B,S,IN], weight_q:int8[OUT,IN], scalar scales and
per-output scale_y:
    acc   = x @ weight_q.T              (int32 semantics, exact)
    out_q = clip(round(acc * scale_x*scale_w/scale_y), -128, 127).astype(int8)
returning (out_q, scale_y).

Sharding: column-parallel over 8 NeuronCores. Every core receives the full
x, a 512-row shard of weight_q, and the matching shard of the fused requant
scale; each core emits out[:, :, shard]. No collectives.

Device-side design (per core):
  - Both matmul operands need the contraction dim (IN) on SBUF partitions,
    but x and W are IN-contiguous in DRAM. We view int8 *pairs* as fp16 and
    use the xbar DMA-transpose (2-byte dtypes only) directly from DRAM.
    Partition p of a transposed chunk then holds k=2p and k=2p+1 interleaved
    along the free dim; a strided int8->bf16 cast (DVE for even k, ACT for
    odd k) de-interleaves into two bf16 operand tiles. The contraction is a
    sum, so the (consistent) k-permutation is harmless.
  - int8 values are exact in bf16, products are exact in the fp32 PSUM
    accumulation (|partial sums| << 2^24), so the GEMM is bit-exact.
  - Epilogue on DVE: multiply by the requant scale (replicated [128,512]),
    round-to-nearest-even via the +/- 1.5*2^23 fp32 trick, clip with
    min/max, and write int8 (conversion of an exact in-range integer).
"""

import numpy as np

import concourse.mybir as mybir
import concourse.tile as tile
from concourse import bacc
from concourse.bass_utils import run_bass_kernel_spmd

P = 128
N_CORES = 8

ROUND_C = 12582912.0  # 1.5 * 2**23: fp32 add/sub forces round-to-nearest-even


def build_nc(t_full, k_full, o_shard, tb, cast_group=1):
    """Build the single-core (SPMD) program.

    t_full: total tokens (B*S), k_full: in_features, o_shard: out features
    per core, tb: tokens per x-slab, cast_group: k-chunks per cast op
    (smaller groups start the PE pipeline earlier).
    """
    assert t_full % tb == 0 and tb % P == 0 and k_full % 256 == 0
    ntb = t_full // tb
    kc = k_full // 256  # 128-partition chunks of k-pairs
    assert kc % cast_group == 0
    steady_group = min(4, kc)  # coarser casts once the pipeline is warm
    assert kc % steady_group == 0
    tsub = tb // P

    nc = bacc.Bacc(
        "TRN2", target_bir_lowering=False, debug=False, num_devices=N_CORES
    )

    xp = nc.dram_tensor(
        "xp", [t_full, k_full // 2], mybir.dt.float16, kind="ExternalInput"
    )
    wp = nc.dram_tensor(
        "wp", [o_shard, k_full // 2], mybir.dt.float16, kind="ExternalInput"
    )
    rs = nc.dram_tensor("rs", [o_shard], mybir.dt.float32, kind="ExternalInput")
    out = nc.dram_tensor("out", [t_full, o_shard], mybir.dt.int8, kind="ExternalOutput")

    xp_ap, wp_ap, rs_ap, out_ap = xp.ap(), wp.ap(), rs.ap(), out.ap()

    with tile.TileContext(nc) as tc:
        with (
            tc.tile_pool(name="const", bufs=1) as const,
            tc.tile_pool(name="wstage", bufs=1) as wstage,
            tc.tile_pool(name="xstage", bufs=2) as xstage,
            tc.tile_pool(name="xbuf", bufs=2) as xbuf,
            tc.tile_pool(name="psum", bufs=4, space="PSUM") as psum,
            tc.tile_pool(name="tmp", bufs=3) as tmp,
            tc.tile_pool(name="obuf", bufs=3) as obuf,
        ):
            def cast_groups(src8, dst0, dst1, g, group):
                """De-interleave one cast-group of k-chunks: even k on DVE,
                odd k on ACT."""
                sl = slice(g * group, (g + 1) * group)
                nc.vector.tensor_copy(dst0[:, sl, :], src8[:, sl, :, 0])
                nc.scalar.copy(dst1[:, sl, :], src8[:, sl, :, 1])

            # ---- weight prep + first x slab, interleaved so the first
            # matmul's dependencies resolve after one cast-group.
            wtr = wstage.tile([P, kc, o_shard], mybir.dt.float16)
            wtr8 = wtr[:].bitcast(mybir.dt.int8).rearrange(
                "p c (m two) -> p c m two", two=2
            )
            wb = [
                const.tile([P, kc, o_shard], mybir.dt.bfloat16, tag=f"wb{b}", name=f"wb{b}")
                for b in range(2)
            ]

            xtr0 = xstage.tile([P, kc, tb], mybir.dt.float16, tag="xtr")
            xtr0_8 = xtr0[:].bitcast(mybir.dt.int8).rearrange(
                "p c (t two) -> p c t two", two=2
            )
            xb0 = [
                xbuf.tile([P, kc, tb], mybir.dt.bfloat16, tag=f"xb{b}", name=f"xb{b}")
                for b in range(2)
            ]
            for c in range(kc):
                nc.sync.dma_start(
                    wtr[:, c, :], wp_ap[:, c * P : (c + 1) * P], transpose=True
                )
                nc.sync.dma_start(
                    xtr0[:, c, :], xp_ap[0:tb, c * P : (c + 1) * P], transpose=True
                )
                if (c + 1) % cast_group == 0:
                    g = c // cast_group
                    cast_groups(wtr8, wb[0], wb[1], g, cast_group)
                    cast_groups(xtr0_8, xb0[0], xb0[1], g, cast_group)

            # requant scale, replicated across partitions (SWDGE queue, and
            # after the prologue so its descriptor-gen doesn't delay the
            # first transposes; first use is the first epilogue ~35us in)
            rs_t = const.tile([P, o_shard], mybir.dt.float32)
            nc.gpsimd.dma_start(rs_t[:], rs_ap[None, :].to_broadcast((P, o_shard)))

            # ---- main loop over token slabs
            for t in range(ntb):
                if t == 0:
                    xb = xb0
                else:
                    # one xbar transpose for the whole slab: contiguous 4KB
                    # source rows, 3D dst [P, kc, tb] (chunk dim is logically
                    # part of the partition dim for the xbar)
                    xtr = xstage.tile([P, kc, tb], mybir.dt.float16, tag="xtr")
                    nc.sync.dma_start(
                        xtr[:], xp_ap[t * tb : (t + 1) * tb, :], transpose=True
                    )
                    xtr8 = xtr[:].bitcast(mybir.dt.int8).rearrange(
                        "p c (t two) -> p c t two", two=2
                    )
                    xb = [
                        xbuf.tile([P, kc, tb], mybir.dt.bfloat16, tag=f"xb{b}", name=f"xb{b}")
                        for b in range(2)
                    ]
                    for g in range(kc // steady_group):
                        cast_groups(xtr8, xb[0], xb[1], g, steady_group)

                def mm(acc, s, c, b, i):
                    nc.tensor.matmul(
                        acc[:],
                        xb[b][:, c, s * P : (s + 1) * P],
                        wb[b][:, c, :],
                        start=(i == 0),
                        stop=(i == 2 * kc - 1),
                    )

                accs = [
                    psum.tile([P, o_shard], mybir.dt.float32, tag="acc",
                              name=f"acc{t}_{s}")
                    for s in range(tsub)
                ]
                if t == 0:
                    # chunk-major: each cast feeds 2*tsub matmuls, so the PE
                    # keeps pace with cast production during warmup
                    for c in range(kc):
                        for b in range(2):
                            for s in range(tsub):
                                mm(accs[s], s, c, b, 2 * c + b)
                else:
                    for s in range(tsub):
                        for c in range(kc):
                            for b in range(2):
                                mm(accs[s], s, c, b, 2 * c + b)

                for s in range(tsub):
                    acc = accs[s]
                    t1 = tmp.tile([P, o_shard], mybir.dt.float32, tag="t1")
                    nc.vector.tensor_tensor(
                        t1[:], acc[:], rs_t[:], mybir.AluOpType.mult
                    )
                    t2 = tmp.tile([P, o_shard], mybir.dt.float32, tag="t2")
                    nc.vector.tensor_scalar(
                        t2[:],
                        t1[:],
                        ROUND_C,
                        ROUND_C,
                        mybir.AluOpType.add,
                        mybir.AluOpType.subtract,
                    )
                    oq = obuf.tile([P, o_shard], mybir.dt.int8, tag="oq")
                    nc.vector.tensor_scalar(
                        oq[:],
                        t2[:],
                        127.0,
                        -128.0,
                        mybir.AluOpType.min,
                        mybir.AluOpType.max,
                    )
                    row = (t * tsub + s) * P
                    nc.scalar.dma_start(out_ap[row : row + P, :], oq[:])

    nc.compile()
    return nc


_NC_CACHE = {}


def _get_nc(key):
    if key not in _NC_CACHE:
        _NC_CACHE[key] = build_nc(*key)
    return _NC_CACHE[key]


def kernel(x, weight_q, scale_x, scale_w, scale_y):
    B, S, K = x.shape
    O = weight_q.shape[0]
    t_full = B * S
    o_shard = O // N_CORES

    x = np.ascontiguousarray(np.asarray(x, dtype=np.int8))
    weight_q = np.ascontiguousarray(np.asarray(weight_q, dtype=np.int8))
    scale_y = np.asarray(scale_y, dtype=np.float32)
    # Fused requant scale via the same jnp ops as the reference, so the
    # division picks up identical bits on whatever jax backend is active.
    import jax.numpy as jnp

    rs_full = np.asarray(
        (
            jnp.asarray(scale_x, jnp.float32)
            * jnp.asarray(scale_w, jnp.float32)
            / jnp.asarray(scale_y, jnp.float32)
        ).astype(jnp.float32)
    )

    xp = x.reshape(t_full, K).view(np.float16)
    in_maps = []
    for c in range(N_CORES):
        wsh = np.ascontiguousarray(
            weight_q[c * o_shard : (c + 1) * o_shard]
        ).view(np.float16)
        in_maps.append(
            {
                "xp": xp,
                "wp": wsh,
                "rs": np.ascontiguousarray(
                    rs_full[c * o_shard : (c + 1) * o_shard]
                ),
            }
        )

    nc = _get_nc((t_full, K, o_shard, 512))
    res = run_bass_kernel_spmd(nc, in_maps, core_ids=list(range(N_CORES)))
    global LAST_RESULTS
    LAST_RESULTS = res
    out = np.concatenate([r["out"] for r in res.results], axis=1)
    return out.reshape(B, S, O).astype(np.int8), scale_y


LAST_RESULTS = None


# revision 27
# speedup vs baseline: 1.0005x; 1.0005x over previous
"""Trainium2 Bass kernel for int8 quantized linear with requantization.

Computes, for x:int8[B,S,IN], weight_q:int8[OUT,IN], scalar scales and
per-output scale_y:
    acc   = x @ weight_q.T              (int32 semantics, exact)
    out_q = clip(round(acc * scale_x*scale_w/scale_y), -128, 127).astype(int8)
returning (out_q, scale_y).

Sharding: column-parallel over 8 NeuronCores. Every core receives the full
x, a 512-row shard of weight_q, and the matching shard of the fused requant
scale; each core emits out[:, :, shard]. No collectives.

Device-side design (per core):
  - Both matmul operands need the contraction dim (IN) on SBUF partitions,
    but x and W are IN-contiguous in DRAM. We view int8 *pairs* as fp16 and
    use the xbar DMA-transpose (2-byte dtypes only) directly from DRAM.
    Partition p of a transposed chunk then holds k=2p and k=2p+1 interleaved
    along the free dim; a strided int8->bf16 cast (DVE for even k, ACT for
    odd k) de-interleaves into two bf16 operand tiles. The contraction is a
    sum, so the (consistent) k-permutation is harmless.
  - int8 values are exact in bf16, products are exact in the fp32 PSUM
    accumulation (|partial sums| << 2^24), so the GEMM is bit-exact.
  - Epilogue on DVE: multiply by the requant scale (replicated [128,512]),
    round-to-nearest-even via the +/- 1.5*2^23 fp32 trick, clip with
    min/max, and write int8 (conversion of an exact in-range integer).
"""

import numpy as np

import concourse.mybir as mybir
import concourse.tile as tile
from concourse import bacc
from concourse.bass_utils import run_bass_kernel_spmd

P = 128
N_CORES = 8

ROUND_C = 12582912.0  # 1.5 * 2**23: fp32 add/sub forces round-to-nearest-even


def build_nc(t_full, k_full, o_shard, tb, cast_group=1):
    """Build the single-core (SPMD) program.

    t_full: total tokens (B*S), k_full: in_features, o_shard: out features
    per core, tb: tokens per x-slab, cast_group: k-chunks per cast op
    (smaller groups start the PE pipeline earlier).
    """
    assert t_full % tb == 0 and tb % P == 0 and k_full % 256 == 0
    ntb = t_full // tb
    kc = k_full // 256  # 128-partition chunks of k-pairs
    assert kc % cast_group == 0
    steady_group = min(4, kc)  # coarser casts once the pipeline is warm
    assert kc % steady_group == 0
    tsub = tb // P

    nc = bacc.Bacc(
        "TRN2", target_bir_lowering=False, debug=False, num_devices=N_CORES
    )

    xp = nc.dram_tensor(
        "xp", [t_full, k_full // 2], mybir.dt.float16, kind="ExternalInput"
    )
    wp = nc.dram_tensor(
        "wp", [o_shard, k_full // 2], mybir.dt.float16, kind="ExternalInput"
    )
    rs = nc.dram_tensor("rs", [o_shard], mybir.dt.float32, kind="ExternalInput")
    out = nc.dram_tensor("out", [t_full, o_shard], mybir.dt.int8, kind="ExternalOutput")

    xp_ap, wp_ap, rs_ap, out_ap = xp.ap(), wp.ap(), rs.ap(), out.ap()

    with tile.TileContext(nc) as tc:
        with (
            tc.tile_pool(name="const", bufs=1) as const,
            tc.tile_pool(name="wstage", bufs=1) as wstage,
            tc.tile_pool(name="xstage", bufs=2) as xstage,
            tc.tile_pool(name="xbuf", bufs=2) as xbuf,
            tc.tile_pool(name="psum", bufs=8, space="PSUM") as psum,
            tc.tile_pool(name="tmp", bufs=4) as tmp,
            tc.tile_pool(name="obuf", bufs=3) as obuf,
        ):
            def cast_groups(src8, dst0, dst1, g, group):
                """De-interleave one cast-group of k-chunks: even k on DVE,
                odd k on ACT."""
                sl = slice(g * group, (g + 1) * group)
                nc.vector.tensor_copy(dst0[:, sl, :], src8[:, sl, :, 0])
                nc.scalar.copy(dst1[:, sl, :], src8[:, sl, :, 1])

            # ---- weight prep + first x slab, interleaved so the first
            # matmul's dependencies resolve after one cast-group.
            wtr = wstage.tile([P, kc, o_shard], mybir.dt.float16)
            wtr8 = wtr[:].bitcast(mybir.dt.int8).rearrange(
                "p c (m two) -> p c m two", two=2
            )
            wb = [
                const.tile([P, kc, o_shard], mybir.dt.bfloat16, tag=f"wb{b}", name=f"wb{b}")
                for b in range(2)
            ]

            xtr0 = xstage.tile([P, kc, tb], mybir.dt.float16, tag="xtr")
            xtr0_8 = xtr0[:].bitcast(mybir.dt.int8).rearrange(
                "p c (t two) -> p c t two", two=2
            )
            xb0 = [
                xbuf.tile([P, kc, tb], mybir.dt.bfloat16, tag=f"xb{b}", name=f"xb{b}")
                for b in range(2)
            ]
            for c in range(kc):
                nc.sync.dma_start(
                    wtr[:, c, :], wp_ap[:, c * P : (c + 1) * P], transpose=True
                )
                nc.sync.dma_start(
                    xtr0[:, c, :], xp_ap[0:tb, c * P : (c + 1) * P], transpose=True
                )
                if (c + 1) % cast_group == 0:
                    g = c // cast_group
                    cast_groups(wtr8, wb[0], wb[1], g, cast_group)
                    cast_groups(xtr0_8, xb0[0], xb0[1], g, cast_group)

            # requant scale, replicated across partitions (SWDGE queue, and
            # after the prologue so its descriptor-gen doesn't delay the
            # first transposes; first use is the first epilogue ~35us in)
            rs_t = const.tile([P, o_shard], mybir.dt.float32)
            nc.gpsimd.dma_start(rs_t[:], rs_ap[None, :].to_broadcast((P, o_shard)))

            # ---- main loop over token slabs
            for t in range(ntb):
                if t == 0:
                    xb = xb0
                else:
                    # one xbar transpose for the whole slab: contiguous 4KB
                    # source rows, 3D dst [P, kc, tb] (chunk dim is logically
                    # part of the partition dim for the xbar)
                    xtr = xstage.tile([P, kc, tb], mybir.dt.float16, tag="xtr")
                    nc.sync.dma_start(
                        xtr[:], xp_ap[t * tb : (t + 1) * tb, :], transpose=True
                    )
                    xtr8 = xtr[:].bitcast(mybir.dt.int8).rearrange(
                        "p c (t two) -> p c t two", two=2
                    )
                    xb = [
                        xbuf.tile([P, kc, tb], mybir.dt.bfloat16, tag=f"xb{b}", name=f"xb{b}")
                        for b in range(2)
                    ]
                    for g in range(kc // steady_group):
                        cast_groups(xtr8, xb[0], xb[1], g, steady_group)

                def mm(acc, s, c, b, i):
                    nc.tensor.matmul(
                        acc[:],
                        xb[b][:, c, s * P : (s + 1) * P],
                        wb[b][:, c, :],
                        start=(i == 0),
                        stop=(i == 2 * kc - 1),
                    )

                accs = [
                    psum.tile([P, o_shard], mybir.dt.float32, tag="acc",
                              name=f"acc{t}_{s}")
                    for s in range(tsub)
                ]
                if t == 0:
                    # chunk-major: each cast feeds 2*tsub matmuls, so the PE
                    # keeps pace with cast production during warmup
                    for c in range(kc):
                        for b in range(2):
                            for s in range(tsub):
                                mm(accs[s], s, c, b, 2 * c + b)
                else:
                    for s in range(tsub):
                        for c in range(kc):
                            for b in range(2):
                                mm(accs[s], s, c, b, 2 * c + b)

                for s in range(tsub):
                    acc = accs[s]
                    t1 = tmp.tile([P, o_shard], mybir.dt.float32, tag="t1")
                    nc.vector.tensor_tensor(
                        t1[:], acc[:], rs_t[:], mybir.AluOpType.mult
                    )
                    t2 = tmp.tile([P, o_shard], mybir.dt.float32, tag="t2")
                    nc.vector.tensor_scalar(
                        t2[:],
                        t1[:],
                        ROUND_C,
                        ROUND_C,
                        mybir.AluOpType.add,
                        mybir.AluOpType.subtract,
                    )
                    oq = obuf.tile([P, o_shard], mybir.dt.int8, tag="oq")
                    nc.vector.tensor_scalar(
                        oq[:],
                        t2[:],
                        127.0,
                        -128.0,
                        mybir.AluOpType.min,
                        mybir.AluOpType.max,
                    )
                    row = (t * tsub + s) * P
                    nc.scalar.dma_start(out_ap[row : row + P, :], oq[:])

    nc.compile()
    return nc


_NC_CACHE = {}


def _get_nc(key):
    if key not in _NC_CACHE:
        _NC_CACHE[key] = build_nc(*key)
    return _NC_CACHE[key]


def kernel(x, weight_q, scale_x, scale_w, scale_y):
    B, S, K = x.shape
    O = weight_q.shape[0]
    t_full = B * S
    o_shard = O // N_CORES

    x = np.ascontiguousarray(np.asarray(x, dtype=np.int8))
    weight_q = np.ascontiguousarray(np.asarray(weight_q, dtype=np.int8))
    scale_y = np.asarray(scale_y, dtype=np.float32)
    # Fused requant scale via the same jnp ops as the reference, so the
    # division picks up identical bits on whatever jax backend is active.
    import jax.numpy as jnp

    rs_full = np.asarray(
        (
            jnp.asarray(scale_x, jnp.float32)
            * jnp.asarray(scale_w, jnp.float32)
            / jnp.asarray(scale_y, jnp.float32)
        ).astype(jnp.float32)
    )

    xp = x.reshape(t_full, K).view(np.float16)
    in_maps = []
    for c in range(N_CORES):
        wsh = np.ascontiguousarray(
            weight_q[c * o_shard : (c + 1) * o_shard]
        ).view(np.float16)
        in_maps.append(
            {
                "xp": xp,
                "wp": wsh,
                "rs": np.ascontiguousarray(
                    rs_full[c * o_shard : (c + 1) * o_shard]
                ),
            }
        )

    nc = _get_nc((t_full, K, o_shard, 512))
    res = run_bass_kernel_spmd(nc, in_maps, core_ids=list(range(N_CORES)))
    global LAST_RESULTS
    LAST_RESULTS = res
    out = np.concatenate([r["out"] for r in res.results], axis=1)
    return out.reshape(B, S, O).astype(np.int8), scale_y


LAST_RESULTS = None


# revision 32
# speedup vs baseline: 1.0036x; 1.0031x over previous
"""Trainium2 Bass kernel for int8 quantized linear with requantization.

Computes, for x:int8[B,S,IN], weight_q:int8[OUT,IN], scalar scales and
per-output scale_y:
    acc   = x @ weight_q.T              (int32 semantics, exact)
    out_q = clip(round(acc * scale_x*scale_w/scale_y), -128, 127).astype(int8)
returning (out_q, scale_y).

Sharding: column-parallel over 8 NeuronCores. Every core receives the full
x, a 512-row shard of weight_q, and the matching shard of the fused requant
scale; each core emits out[:, :, shard]. No collectives.

Device-side design (per core):
  - Both matmul operands need the contraction dim (IN) on SBUF partitions,
    but x and W are IN-contiguous in DRAM. We view int8 *pairs* as fp16 and
    use the xbar DMA-transpose (2-byte dtypes only) directly from DRAM.
    Partition p of a transposed chunk then holds k=2p and k=2p+1 interleaved
    along the free dim; a strided int8->bf16 cast (DVE for even k, ACT for
    odd k) de-interleaves into two bf16 operand tiles. The contraction is a
    sum, so the (consistent) k-permutation is harmless.
  - int8 values are exact in bf16, products are exact in the fp32 PSUM
    accumulation (|partial sums| << 2^24), so the GEMM is bit-exact.
  - Epilogue on DVE: multiply by the requant scale (replicated [128,512]),
    round-to-nearest-even via the +/- 1.5*2^23 fp32 trick, clip with
    min/max, and write int8 (conversion of an exact in-range integer).
"""

import numpy as np

import concourse.mybir as mybir
import concourse.tile as tile
from concourse import bacc
from concourse.bass_utils import run_bass_kernel_spmd

P = 128
N_CORES = 8

ROUND_C = 12582912.0  # 1.5 * 2**23: fp32 add/sub forces round-to-nearest-even


def build_nc(t_full, k_full, o_shard, tb, cast_group=1):
    """Build the single-core (SPMD) program.

    t_full: total tokens (B*S), k_full: in_features, o_shard: out features
    per core, tb: tokens per x-slab, cast_group: k-chunks per cast op
    (smaller groups start the PE pipeline earlier).
    """
    assert t_full % tb == 0 and tb % P == 0 and k_full % 256 == 0
    ntb = t_full // tb
    kc = k_full // 256  # 128-partition chunks of k-pairs
    assert kc % cast_group == 0
    steady_group = min(4, kc)  # coarser casts once the pipeline is warm
    assert kc % steady_group == 0
    tsub = tb // P

    nc = bacc.Bacc(
        "TRN2", target_bir_lowering=False, debug=False, num_devices=N_CORES
    )

    xp = nc.dram_tensor(
        "xp", [t_full, k_full // 2], mybir.dt.float16, kind="ExternalInput"
    )
    wp = nc.dram_tensor(
        "wp", [o_shard, k_full // 2], mybir.dt.float16, kind="ExternalInput"
    )
    rs = nc.dram_tensor("rs", [o_shard], mybir.dt.float32, kind="ExternalInput")
    out = nc.dram_tensor("out", [t_full, o_shard], mybir.dt.int8, kind="ExternalOutput")

    xp_ap, wp_ap, rs_ap, out_ap = xp.ap(), wp.ap(), rs.ap(), out.ap()

    with tile.TileContext(nc) as tc:
        with (
            tc.tile_pool(name="const", bufs=1) as const,
            tc.tile_pool(name="wstage", bufs=1) as wstage,
            tc.tile_pool(name="xstage", bufs=2) as xstage,
            tc.tile_pool(name="xbuf", bufs=2) as xbuf,
            tc.tile_pool(name="psum", bufs=6, space="PSUM") as psum,
            tc.tile_pool(name="psumh", bufs=2, space="PSUM") as psumh,
            tc.tile_pool(name="tmp", bufs=4) as tmp,
            tc.tile_pool(name="obuf", bufs=3) as obuf,
        ):
            def cast_groups(src8, dst0, dst1, g, group):
                """De-interleave one cast-group of k-chunks: even k on DVE,
                odd k on ACT."""
                sl = slice(g * group, (g + 1) * group)
                nc.vector.tensor_copy(dst0[:, sl, :], src8[:, sl, :, 0])
                nc.scalar.copy(dst1[:, sl, :], src8[:, sl, :, 1])

            # warm up the HWDGE rings: tiny DMAs absorb the DGE init
            # latency before the first (critical-path) transposes
            warm_t = wstage.tile([1, 16], mybir.dt.float32, name="warm_t")
            nc.sync.dma_start(warm_t[:, 0:8], rs_ap[0:8][None, :])
            nc.scalar.dma_start(warm_t[:, 8:16], rs_ap[0:8][None, :])

            # ---- weight prep + first x slab, interleaved so the first
            # matmul's dependencies resolve after one cast-group.
            wtr = wstage.tile([P, kc, o_shard], mybir.dt.float16)
            wtr8 = wtr[:].bitcast(mybir.dt.int8).rearrange(
                "p c (m two) -> p c m two", two=2
            )
            wb = [
                const.tile([P, kc, o_shard], mybir.dt.bfloat16, tag=f"wb{b}", name=f"wb{b}")
                for b in range(2)
            ]

            xtr0 = xstage.tile([P, kc, tb], mybir.dt.float16, tag="xtr")
            xtr0_8 = xtr0[:].bitcast(mybir.dt.int8).rearrange(
                "p c (t two) -> p c t two", two=2
            )
            xb0 = [
                xbuf.tile([P, kc, tb], mybir.dt.bfloat16, tag=f"xb{b}", name=f"xb{b}")
                for b in range(2)
            ]
            for c in range(kc):
                nc.sync.dma_start(
                    wtr[:, c, :], wp_ap[:, c * P : (c + 1) * P], transpose=True
                )
                nc.sync.dma_start(
                    xtr0[:, c, :], xp_ap[0:tb, c * P : (c + 1) * P], transpose=True
                )
                if (c + 1) % cast_group == 0:
                    g = c // cast_group
                    cast_groups(wtr8, wb[0], wb[1], g, cast_group)
                    cast_groups(xtr0_8, xb0[0], xb0[1], g, cast_group)

            # requant scale, replicated across partitions (SWDGE queue, and
            # after the prologue so its descriptor-gen doesn't delay the
            # first transposes; first use is the first epilogue ~35us in)
            rs_t = const.tile([P, o_shard], mybir.dt.float32)
            nc.gpsimd.dma_start(rs_t[:], rs_ap[None, :].to_broadcast((P, o_shard)))

            # ---- main loop over token slabs
            for t in range(ntb):
                if t == 0:
                    xb = xb0
                else:
                    # one xbar transpose for the whole slab: contiguous 4KB
                    # source rows, 3D dst [P, kc, tb] (chunk dim is logically
                    # part of the partition dim for the xbar)
                    xtr = xstage.tile([P, kc, tb], mybir.dt.float16, tag="xtr")
                    nc.sync.dma_start(
                        xtr[:], xp_ap[t * tb : (t + 1) * tb, :], transpose=True
                    )
                    xtr8 = xtr[:].bitcast(mybir.dt.int8).rearrange(
                        "p c (t two) -> p c t two", two=2
                    )
                    xb = [
                        xbuf.tile([P, kc, tb], mybir.dt.bfloat16, tag=f"xb{b}", name=f"xb{b}")
                        for b in range(2)
                    ]
                    for g in range(kc // steady_group):
                        cast_groups(xtr8, xb[0], xb[1], g, steady_group)

                def mm(acc, s, c, b, i):
                    nc.tensor.matmul(
                        acc[:],
                        xb[b][:, c, s * P : (s + 1) * P],
                        wb[b][:, c, :],
                        start=(i == 0),
                        stop=(i == 2 * kc - 1),
                    )

                # on the last slab the final s-group is emitted separately
                # as two o-halves (below)
                nsub = tsub - 1 if t == ntb - 1 else tsub
                accs = [
                    psum.tile([P, o_shard], mybir.dt.float32, tag="acc",
                              name=f"acc{t}_{s}")
                    for s in range(nsub)
                ]
                if t == 0:
                    # chunk-major: each cast feeds 2*tsub matmuls, so the PE
                    # keeps pace with cast production during warmup
                    for c in range(kc):
                        for b in range(2):
                            for s in range(nsub):
                                mm(accs[s], s, c, b, 2 * c + b)
                else:
                    for s in range(nsub):
                        for c in range(kc):
                            for b in range(2):
                                mm(accs[s], s, c, b, 2 * c + b)

                if t == ntb - 1:
                    # split the very last psum group into two o-halves so the
                    # final exposed epilogue (after the last matmul) is half
                    # the size; half-0's epilogue overlaps half-1's matmuls
                    half = o_shard // 2
                    s = tsub - 1
                    for h in range(2):
                        acch = psumh.tile([P, half], mybir.dt.float32,
                                         tag="acch", name=f"acch{h}")
                        for c in range(kc):
                            for b in range(2):
                                i = 2 * c + b
                                nc.tensor.matmul(
                                    acch[:],
                                    xb[b][:, c, s * P : (s + 1) * P],
                                    wb[b][:, c, h * half : (h + 1) * half],
                                    start=(i == 0),
                                    stop=(i == 2 * kc - 1),
                                )
                        t1h = tmp.tile([P, half], mybir.dt.float32, tag="t1h")
                        nc.vector.tensor_tensor(
                            t1h[:], acch[:], rs_t[:, h * half : (h + 1) * half],
                            mybir.AluOpType.mult,
                        )
                        t2h = tmp.tile([P, half], mybir.dt.float32, tag="t2h")
                        nc.vector.tensor_scalar(
                            t2h[:], t1h[:], ROUND_C, ROUND_C,
                            mybir.AluOpType.add, mybir.AluOpType.subtract,
                        )
                        oqh = obuf.tile([P, half], mybir.dt.int8, tag="oqh")
                        nc.vector.tensor_scalar(
                            oqh[:], t2h[:], 127.0, -128.0,
                            mybir.AluOpType.min, mybir.AluOpType.max,
                        )
                        row = (t * tsub + s) * P
                        nc.scalar.dma_start(
                            out_ap[row : row + P, h * half : (h + 1) * half],
                            oqh[:],
                        )

                for s in range(len(accs)):
                    acc = accs[s]
                    t1 = tmp.tile([P, o_shard], mybir.dt.float32, tag="t1")
                    nc.vector.tensor_tensor(
                        t1[:], acc[:], rs_t[:], mybir.AluOpType.mult
                    )
                    t2 = tmp.tile([P, o_shard], mybir.dt.float32, tag="t2")
                    nc.vector.tensor_scalar(
                        t2[:],
                        t1[:],
                        ROUND_C,
                        ROUND_C,
                        mybir.AluOpType.add,
                        mybir.AluOpType.subtract,
                    )
                    oq = obuf.tile([P, o_shard], mybir.dt.int8, tag="oq")
                    nc.vector.tensor_scalar(
                        oq[:],
                        t2[:],
                        127.0,
                        -128.0,
                        mybir.AluOpType.min,
                        mybir.AluOpType.max,
                    )
                    row = (t * tsub + s) * P
                    nc.scalar.dma_start(out_ap[row : row + P, :], oq[:])

    nc.compile()
    return nc


_NC_CACHE = {}


def _get_nc(key):
    if key not in _NC_CACHE:
        _NC_CACHE[key] = build_nc(*key)
    return _NC_CACHE[key]


def kernel(x, weight_q, scale_x, scale_w, scale_y):
    B, S, K = x.shape
    O = weight_q.shape[0]
    t_full = B * S
    o_shard = O // N_CORES

    x = np.ascontiguousarray(np.asarray(x, dtype=np.int8))
    weight_q = np.ascontiguousarray(np.asarray(weight_q, dtype=np.int8))
    scale_y = np.asarray(scale_y, dtype=np.float32)
    # Fused requant scale via the same jnp ops as the reference, so the
    # division picks up identical bits on whatever jax backend is active.
    import jax.numpy as jnp

    rs_full = np.asarray(
        (
            jnp.asarray(scale_x, jnp.float32)
            * jnp.asarray(scale_w, jnp.float32)
            / jnp.asarray(scale_y, jnp.float32)
        ).astype(jnp.float32)
    )

    xp = x.reshape(t_full, K).view(np.float16)
    in_maps = []
    for c in range(N_CORES):
        wsh = np.ascontiguousarray(
            weight_q[c * o_shard : (c + 1) * o_shard]
        ).view(np.float16)
        in_maps.append(
            {
                "xp": xp,
                "wp": wsh,
                "rs": np.ascontiguousarray(
                    rs_full[c * o_shard : (c + 1) * o_shard]
                ),
            }
        )

    nc = _get_nc((t_full, K, o_shard, 512))
    res = run_bass_kernel_spmd(nc, in_maps, core_ids=list(range(N_CORES)))
    global LAST_RESULTS
    LAST_RESULTS = res
    out = np.concatenate([r["out"] for r in res.results], axis=1)
    return out.reshape(B, S, O).astype(np.int8), scale_y


LAST_RESULTS = None


# revision 34
# speedup vs baseline: 1.0137x; 1.0100x over previous
"""Trainium2 Bass kernel for int8 quantized linear with requantization.

Computes, for x:int8[B,S,IN], weight_q:int8[OUT,IN], scalar scales and
per-output scale_y:
    acc   = x @ weight_q.T              (int32 semantics, exact)
    out_q = clip(round(acc * scale_x*scale_w/scale_y), -128, 127).astype(int8)
returning (out_q, scale_y).

Sharding: column-parallel over 8 NeuronCores. Every core receives the full
x, a 512-row shard of weight_q, and the matching shard of the fused requant
scale; each core emits out[:, :, shard]. No collectives.

Device-side design (per core):
  - Both matmul operands need the contraction dim (IN) on SBUF partitions,
    but x and W are IN-contiguous in DRAM. We view int8 *pairs* as fp16 and
    use the xbar DMA-transpose (2-byte dtypes only) directly from DRAM.
    Partition p of a transposed chunk then holds k=2p and k=2p+1 interleaved
    along the free dim; a strided int8->bf16 cast (DVE for even k, ACT for
    odd k) de-interleaves into two bf16 operand tiles. The contraction is a
    sum, so the (consistent) k-permutation is harmless.
  - int8 values are exact in bf16, products are exact in the fp32 PSUM
    accumulation (|partial sums| << 2^24), so the GEMM is bit-exact.
  - Epilogue on DVE: multiply by the requant scale (replicated [128,512]),
    round-to-nearest-even via the +/- 1.5*2^23 fp32 trick, clip with
    min/max, and write int8 (conversion of an exact in-range integer).
"""

import numpy as np

import concourse.mybir as mybir
import concourse.tile as tile
from concourse import bacc
from concourse.bass_utils import run_bass_kernel_spmd

P = 128
N_CORES = 8

ROUND_C = 12582912.0  # 1.5 * 2**23: fp32 add/sub forces round-to-nearest-even


def build_nc(t_full, k_full, o_shard, tb, cast_group=1):
    """Build the single-core (SPMD) program.

    t_full: total tokens (B*S), k_full: in_features, o_shard: out features
    per core, tb: tokens per x-slab, cast_group: k-chunks per cast op
    (smaller groups start the PE pipeline earlier).
    """
    assert t_full % tb == 0 and tb % P == 0 and k_full % 256 == 0
    ntb = t_full // tb
    kc = k_full // 256  # 128-partition chunks of k-pairs
    assert kc % cast_group == 0
    steady_group = min(4, kc)  # coarser casts once the pipeline is warm
    assert kc % steady_group == 0
    tsub = tb // P

    nc = bacc.Bacc(
        "TRN2", target_bir_lowering=False, debug=False, num_devices=N_CORES
    )

    xp = nc.dram_tensor(
        "xp", [t_full, k_full // 2], mybir.dt.float16, kind="ExternalInput"
    )
    wp = nc.dram_tensor(
        "wp", [o_shard, k_full // 2], mybir.dt.float16, kind="ExternalInput"
    )
    rs = nc.dram_tensor("rs", [o_shard], mybir.dt.float32, kind="ExternalInput")
    out = nc.dram_tensor("out", [t_full, o_shard], mybir.dt.int8, kind="ExternalOutput")

    xp_ap, wp_ap, rs_ap, out_ap = xp.ap(), wp.ap(), rs.ap(), out.ap()

    with tile.TileContext(nc) as tc:
        with (
            tc.tile_pool(name="const", bufs=1) as const,
            tc.tile_pool(name="wstage", bufs=1) as wstage,
            tc.tile_pool(name="xstage", bufs=2) as xstage,
            tc.tile_pool(name="xbuf", bufs=2) as xbuf,
            tc.tile_pool(name="psum", bufs=6, space="PSUM") as psum,
            tc.tile_pool(name="psumh", bufs=2, space="PSUM") as psumh,
            tc.tile_pool(name="tmp", bufs=4) as tmp,
            tc.tile_pool(name="obuf", bufs=3) as obuf,
        ):
            def cast_groups(src8, dst0, dst1, g, group):
                """De-interleave one cast-group of k-chunks: even k on DVE,
                odd k on ACT."""
                sl = slice(g * group, (g + 1) * group)
                nc.vector.tensor_copy(dst0[:, sl, :], src8[:, sl, :, 0])
                nc.scalar.copy(dst1[:, sl, :], src8[:, sl, :, 1])

            # warm up the HWDGE rings: tiny DMAs absorb the DGE init
            # latency before the first (critical-path) transposes
            warm_t = wstage.tile([1, 16], mybir.dt.float32, name="warm_t")
            nc.sync.dma_start(warm_t[:, 0:8], rs_ap[0:8][None, :])
            nc.scalar.dma_start(warm_t[:, 8:16], rs_ap[0:8][None, :])

            # warm up the PE while it would otherwise idle through the
            # prologue: ~3.5us of zero matmuls trips the HAM activity
            # window so the first real matmuls run at 2.4 GHz instead of
            # 1.2 (the cost model doesn't simulate HAM; this is for HW)
            wz = const.tile([P, P], mybir.dt.bfloat16, name="wz")
            xz = const.tile([P, o_shard], mybir.dt.bfloat16, name="xz")
            nc.vector.memset(wz[:], 0.0)
            nc.vector.memset(xz[:], 0.0)
            pe_warm = psum.tile([P, o_shard], mybir.dt.float32, tag="acc",
                                name="pe_warm")
            for i in range(8):
                nc.tensor.matmul(
                    pe_warm[:], wz[:], xz[:], start=(i == 0), stop=(i == 7)
                )

            # ---- weight prep + first x slab, interleaved so the first
            # matmul's dependencies resolve after one cast-group.
            wtr = wstage.tile([P, kc, o_shard], mybir.dt.float16)
            wtr8 = wtr[:].bitcast(mybir.dt.int8).rearrange(
                "p c (m two) -> p c m two", two=2
            )
            wb = [
                const.tile([P, kc, o_shard], mybir.dt.bfloat16, tag=f"wb{b}", name=f"wb{b}")
                for b in range(2)
            ]

            xtr0 = xstage.tile([P, kc, tb], mybir.dt.float16, tag="xtr")
            xtr0_8 = xtr0[:].bitcast(mybir.dt.int8).rearrange(
                "p c (t two) -> p c t two", two=2
            )
            xb0 = [
                xbuf.tile([P, kc, tb], mybir.dt.bfloat16, tag=f"xb{b}", name=f"xb{b}")
                for b in range(2)
            ]
            for c in range(kc):
                nc.sync.dma_start(
                    wtr[:, c, :], wp_ap[:, c * P : (c + 1) * P], transpose=True
                )
                nc.sync.dma_start(
                    xtr0[:, c, :], xp_ap[0:tb, c * P : (c + 1) * P], transpose=True
                )
                if (c + 1) % cast_group == 0:
                    g = c // cast_group
                    cast_groups(wtr8, wb[0], wb[1], g, cast_group)
                    cast_groups(xtr0_8, xb0[0], xb0[1], g, cast_group)

            # requant scale, replicated across partitions (SWDGE queue, and
            # after the prologue so its descriptor-gen doesn't delay the
            # first transposes; first use is the first epilogue ~35us in)
            rs_t = const.tile([P, o_shard], mybir.dt.float32)
            nc.gpsimd.dma_start(rs_t[:], rs_ap[None, :].to_broadcast((P, o_shard)))

            # ---- main loop over token slabs
            for t in range(ntb):
                if t == 0:
                    xb = xb0
                else:
                    # one xbar transpose for the whole slab: contiguous 4KB
                    # source rows, 3D dst [P, kc, tb] (chunk dim is logically
                    # part of the partition dim for the xbar)
                    xtr = xstage.tile([P, kc, tb], mybir.dt.float16, tag="xtr")
                    nc.sync.dma_start(
                        xtr[:], xp_ap[t * tb : (t + 1) * tb, :], transpose=True
                    )
                    xtr8 = xtr[:].bitcast(mybir.dt.int8).rearrange(
                        "p c (t two) -> p c t two", two=2
                    )
                    xb = [
                        xbuf.tile([P, kc, tb], mybir.dt.bfloat16, tag=f"xb{b}", name=f"xb{b}")
                        for b in range(2)
                    ]
                    for g in range(kc // steady_group):
                        cast_groups(xtr8, xb[0], xb[1], g, steady_group)

                def mm(acc, s, c, b, i):
                    nc.tensor.matmul(
                        acc[:],
                        xb[b][:, c, s * P : (s + 1) * P],
                        wb[b][:, c, :],
                        start=(i == 0),
                        stop=(i == 2 * kc - 1),
                    )

                # on the last slab the final s-group is emitted separately
                # as two o-halves (below)
                nsub = tsub - 1 if t == ntb - 1 else tsub
                accs = [
                    psum.tile([P, o_shard], mybir.dt.float32, tag="acc",
                              name=f"acc{t}_{s}")
                    for s in range(nsub)
                ]
                if t == 0:
                    # chunk-major: each cast feeds 2*tsub matmuls, so the PE
                    # keeps pace with cast production during warmup
                    for c in range(kc):
                        for b in range(2):
                            for s in range(nsub):
                                mm(accs[s], s, c, b, 2 * c + b)
                else:
                    for s in range(nsub):
                        for c in range(kc):
                            for b in range(2):
                                mm(accs[s], s, c, b, 2 * c + b)

                if t == ntb - 1:
                    # split the very last psum group into two o-halves so the
                    # final exposed epilogue (after the last matmul) is half
                    # the size; half-0's epilogue overlaps half-1's matmuls
                    half = o_shard // 2
                    s = tsub - 1
                    for h in range(2):
                        acch = psumh.tile([P, half], mybir.dt.float32,
                                         tag="acch", name=f"acch{h}")
                        for c in range(kc):
                            for b in range(2):
                                i = 2 * c + b
                                nc.tensor.matmul(
                                    acch[:],
                                    xb[b][:, c, s * P : (s + 1) * P],
                                    wb[b][:, c, h * half : (h + 1) * half],
                                    start=(i == 0),
                                    stop=(i == 2 * kc - 1),
                                )
                        t1h = tmp.tile([P, half], mybir.dt.float32, tag="t1h")
                        nc.vector.tensor_tensor(
                            t1h[:], acch[:], rs_t[:, h * half : (h + 1) * half],
                            mybir.AluOpType.mult,
                        )
                        t2h = tmp.tile([P, half], mybir.dt.float32, tag="t2h")
                        nc.vector.tensor_scalar(
                            t2h[:], t1h[:], ROUND_C, ROUND_C,
                            mybir.AluOpType.add, mybir.AluOpType.subtract,
                        )
                        oqh = obuf.tile([P, half], mybir.dt.int8, tag="oqh")
                        nc.vector.tensor_scalar(
                            oqh[:], t2h[:], 127.0, -128.0,
                            mybir.AluOpType.min, mybir.AluOpType.max,
                        )
                        row = (t * tsub + s) * P
                        nc.scalar.dma_start(
                            out_ap[row : row + P, h * half : (h + 1) * half],
                            oqh[:],
                        )

                for s in range(len(accs)):
                    acc = accs[s]
                    t1 = tmp.tile([P, o_shard], mybir.dt.float32, tag="t1")
                    nc.vector.tensor_tensor(
                        t1[:], acc[:], rs_t[:], mybir.AluOpType.mult
                    )
                    t2 = tmp.tile([P, o_shard], mybir.dt.float32, tag="t2")
                    nc.vector.tensor_scalar(
                        t2[:],
                        t1[:],
                        ROUND_C,
                        ROUND_C,
                        mybir.AluOpType.add,
                        mybir.AluOpType.subtract,
                    )
                    oq = obuf.tile([P, o_shard], mybir.dt.int8, tag="oq")
                    nc.vector.tensor_scalar(
                        oq[:],
                        t2[:],
                        127.0,
                        -128.0,
                        mybir.AluOpType.min,
                        mybir.AluOpType.max,
                    )
                    row = (t * tsub + s) * P
                    nc.scalar.dma_start(out_ap[row : row + P, :], oq[:])

    nc.compile()
    return nc


_NC_CACHE = {}


def _get_nc(key):
    if key not in _NC_CACHE:
        _NC_CACHE[key] = build_nc(*key)
    return _NC_CACHE[key]


def kernel(x, weight_q, scale_x, scale_w, scale_y):
    B, S, K = x.shape
    O = weight_q.shape[0]
    t_full = B * S
    o_shard = O // N_CORES

    x = np.ascontiguousarray(np.asarray(x, dtype=np.int8))
    weight_q = np.ascontiguousarray(np.asarray(weight_q, dtype=np.int8))
    scale_y = np.asarray(scale_y, dtype=np.float32)
    # Fused requant scale via the same jnp ops as the reference, so the
    # division picks up identical bits on whatever jax backend is active.
    import jax.numpy as jnp

    rs_full = np.asarray(
        (
            jnp.asarray(scale_x, jnp.float32)
            * jnp.asarray(scale_w, jnp.float32)
            / jnp.asarray(scale_y, jnp.float32)
        ).astype(jnp.float32)
    )

    xp = x.reshape(t_full, K).view(np.float16)
    in_maps = []
    for c in range(N_CORES):
        wsh = np.ascontiguousarray(
            weight_q[c * o_shard : (c + 1) * o_shard]
        ).view(np.float16)
        in_maps.append(
            {
                "xp": xp,
                "wp": wsh,
                "rs": np.ascontiguousarray(
                    rs_full[c * o_shard : (c + 1) * o_shard]
                ),
            }
        )

    nc = _get_nc((t_full, K, o_shard, 512))
    res = run_bass_kernel_spmd(nc, in_maps, core_ids=list(range(N_CORES)))
    global LAST_RESULTS
    LAST_RESULTS = res
    out = np.concatenate([r["out"] for r in res.results], axis=1)
    return out.reshape(B, S, O).astype(np.int8), scale_y


LAST_RESULTS = None
